# revision 29
# baseline (speedup 1.0000x reference)
"""GraphConv (DGL norm='both' + relu) Trainium2 kernel, 8-core SPMD.

out = relu( D_dst^{-1/2} A D_src^{-1/2} X W + b )

Strategy (per sharding hint): nodes are partitioned across the 8 cores;
edges are partitioned by destination node so the segment-sum scatter is
device-local; x is replicated so source features are gathered directly
from HBM (the "all-gather" is done at input-distribution time); W/b are
replicated.

Device algorithm, per 128-node block:
  - indirect-DMA gather of the block's edge source rows H [128e, 512]
  - build the one-hot scatter matrix S_w[e, n] = (dstloc[e]==n)*nsrc[e]
    on DVE (iota + fused is_equal/mult)
  - PE: agg = S_w.T @ H accumulated over edge chunks  (the segment sum)
  - ACT: agg_sb = agg * ndst  (PSUM->SBUF copy with per-partition scale)
  - PE: 4x 128x128 transposes -> aggT (feats-on-partitions for the GEMM)
  - PE: out = aggT.T @ W + b (bias via a K=1 ones-matmul), ACT relu
Matmuls run in float32r (TF32-like, 4x the fp32 rate); DRAM tensors are
declared float32r and carry raw fp32 bytes (PE rounds internally).

Host does only index-space preprocessing: degree counts (bincount),
balanced node->block assignment, edge bucketing/padding, and the final
inverse permutation of the output rows.
"""

import heapq
import os
import sys

import numpy as np

sys.path.insert(0, "/opt/trn_rl_repo")

P = 128          # partitions / nodes per block
N_CORES = 8
F_IN = 512
F_OUT = 512
K_CH = F_IN // P  # 4 contraction chunks in the GEMM
GATHER_DTYPE = "f32r"  # "f32r" | "fp16" | "bf16"

_PROGRAM_CACHE = {}


# ----------------------------------------------------------------------------
# host-side graph preprocessing (index-space only)
# ----------------------------------------------------------------------------

def _preprocess(src, dst, n_nodes):
    """Balanced node->block assignment + edge bucketing.

    Returns dict with per-core index arrays and the node permutation.
    """
    E = src.shape[0]
    bpc = int(np.ceil(n_nodes / (N_CORES * P)))      # blocks per core
    nblocks = N_CORES * bpc
    npad = nblocks * P

    deg_out = np.bincount(src, minlength=n_nodes).astype(np.int64)
    deg_in = np.bincount(dst, minlength=n_nodes).astype(np.int64)

    # Greedy balanced assignment of nodes to blocks (minimize max block
    # in-edge count so every block needs the same number of edge chunks).
    order = np.argsort(-deg_in, kind="stable")
    block_of = np.empty(n_nodes, np.int64)
    slot_of = np.empty(n_nodes, np.int64)
    counts = np.zeros(nblocks, np.int64)
    heap = [(0, b) for b in range(nblocks)]
    heapq.heapify(heap)
    deg_in_l = deg_in.tolist()
    for n in order.tolist():
        load, b = heapq.heappop(heap)
        block_of[n] = b
        slot_of[n] = counts[b]
        counts[b] += 1
        if counts[b] < P:
            heapq.heappush(heap, (load + deg_in_l[n], b))

    node_order = np.full(npad, -1, np.int64)
    node_order[block_of * P + slot_of] = np.arange(n_nodes)

    # Edge bucketing by destination block.
    eblk = block_of[dst]
    ec = np.bincount(eblk, minlength=nblocks)
    C = max(1, int(np.ceil(ec.max() / P)))           # chunks per block
    ek = np.lexsort((src, eblk))                      # group by block, then src
    eblk_s = eblk[ek]
    starts = np.concatenate(([0], np.cumsum(ec)))
    pos = np.arange(E) - starts[eblk_s]
    chunk = pos // P
    part = pos % P
    core = eblk_s // bpc
    blk_in_core = eblk_s % bpc
    col = blk_in_core * C + chunk

    ncols = bpc * C
    src_idx = np.zeros((N_CORES, P, ncols), np.int32)
    dstloc = np.full((N_CORES, P, ncols), -1.0, np.float32)
    degsrc = np.ones((N_CORES, P, ncols), np.float32)
    src_idx[core, part, col] = src[ek].astype(np.int32)
    dstloc[core, part, col] = slot_of[dst[ek]].astype(np.float32)
    degsrc[core, part, col] = deg_out[src[ek]].astype(np.float32)

    # per-node in-degree, laid out [core, slot(partition), block]
    deg_in_pad = np.ones(npad, np.float32)
    valid = node_order >= 0
    d = deg_in[node_order[valid]]
    deg_in_pad[valid] = np.where(d > 0, d, 1).astype(np.float32)
    degin = np.transpose(deg_in_pad.reshape(N_CORES, bpc, P), (0, 2, 1)).copy()

    return dict(
        bpc=bpc, C=C, npad=npad, node_order=node_order,
        src_idx=src_idx, dstloc=dstloc, degsrc=degsrc, degin=degin,
    )


# ----------------------------------------------------------------------------
# device program
# ----------------------------------------------------------------------------

def _indirect_gather_q(eng, out, in_, offset_ap, queue):
    """nc.gpsimd.indirect_dma_start (gather form), with a SWDGE queue choice.

    Replicates bass.BassGpSimd.indirect_dma_start's lowering but emits the
    InstDMACopy on qPoolDynamic{queue} so gathers can spread across multiple
    SWDGE contexts.
    """
    import concourse.mybir as mybir

    out_ap = eng.lower_ap_dma(out, for_indirect_dma=True)
    in_ap = eng.lower_ap_dma(in_, for_indirect_dma=True)
    assert len(in_ap) == 1 and len(out_ap) == 1
    offset_l = eng.lower_ap_dma(offset_ap)
    assert len(offset_l) == 1
    in_ap.append(offset_l[0])

    ap_shape = in_.shape
    coef = 1
    for i in range(1, len(ap_shape)):
        coef *= ap_shape[i]
    in_ap[0].dynamic_ap_info = mybir.DynamicAccessPatternInfo(
        c=0,
        actual_ap=out.ap,
        indirect_dim_max_index=ap_shape[0],
        offset_expr=[
            mybir.DynamicAccessPatternOffsetExpr(
                coef=coef,
                aff_expr=mybir.DynamicAccessPatternOffsetExprAffExpr(
                    kind="IndirectArgId", arg_id=1),
            )
        ],
    )
    return eng.add_instruction(
        mybir.InstDMACopy(
            name=eng.bass.get_next_instruction_name(),
            queue=f"qPoolDynamic{queue or ''}",
            mode="Copy",
            ins=in_ap,
            outs=out_ap,
            oob_is_err=True,
            cce_op=mybir.AluOpType.bypass,
        )
    )


def _build_program(n_nodes, bpc, C, repeat=1, ablate=(), n_queues=1,
                   gather_dtype="f32r"):
    import concourse.bass as bass
    import concourse.tile as tile
    from concourse import bacc, mybir
    from concourse.masks import make_identity

    ablate = set(ablate)

    f32 = mybir.dt.float32
    f32r = mybir.dt.float32r
    i32 = mybir.dt.int32
    AF = mybir.ActivationFunctionType
    ALU = mybir.AluOpType
    gdt = {"f32r": f32r, "fp16": mybir.dt.float16,
           "bf16": mybir.dt.bfloat16}[gather_dtype]
    # 16-bit gather -> run the whole matmul pipeline (W, agg, transposes) in
    # the same 16-bit dtype: FWL fast weight loads + 1 cyc/row transposes.
    mdt = gdt if gather_dtype != "f32r" else f32r
    tdt = gdt if gather_dtype != "f32r" else f32  # transpose dtype

    ncols = bpc * C

    nc = bacc.Bacc("TRN2", target_bir_lowering=False, debug=False,
                   num_devices=N_CORES, num_swdge_queues=max(1, n_queues))

    x_d = nc.dram_tensor("x", [n_nodes, F_IN], gdt, kind="ExternalInput").ap()
    w_d = nc.dram_tensor("w", [F_IN, F_OUT], mdt, kind="ExternalInput").ap()
    b_d = nc.dram_tensor("b", [1, F_OUT], mdt, kind="ExternalInput").ap()
    ones_d = nc.dram_tensor("ones", [1, P], mdt, kind="ExternalInput").ap()
    srcidx_d = nc.dram_tensor("src_idx", [P, ncols], i32, kind="ExternalInput").ap()
    dstloc_d = nc.dram_tensor("dstloc", [P, ncols], f32, kind="ExternalInput").ap()
    degsrc_d = nc.dram_tensor("degsrc", [P, ncols], f32, kind="ExternalInput").ap()
    degin_d = nc.dram_tensor("degin", [P, bpc], f32, kind="ExternalInput").ap()
    out_d = nc.dram_tensor("out", [bpc * P, F_OUT], f32, kind="ExternalOutput").ap()

    with tile.TileContext(nc) as tc:
        with (
            tc.tile_pool(name="const", bufs=1) as cpool,
            tc.tile_pool(name="gpool", bufs=12) as gpool,
            tc.tile_pool(name="spool", bufs=12) as spool,
            tc.tile_pool(name="apool", bufs=3) as apool,
            tc.tile_pool(name="tpool", bufs=8) as tpool,
            tc.tile_pool(name="opool", bufs=3) as opool,
            tc.tile_pool(name="ps_agg", bufs=2, space="PSUM") as ps_agg,
            tc.tile_pool(name="ps_t", bufs=2, space="PSUM") as ps_t,
            tc.tile_pool(name="ps_out", bufs=2, space="PSUM") as ps_out,
        ):
            # ---- prologue: constants and index arrays
            w_t = cpool.tile([P, K_CH * F_OUT], mdt, tag="w")
            for k in range(K_CH):
                nc.sync.dma_start(
                    w_t[:, k * F_OUT:(k + 1) * F_OUT],
                    w_d[k * P:(k + 1) * P, :])
            b_t = cpool.tile([1, F_OUT], mdt, tag="b")
            nc.sync.dma_start(b_t[:], b_d[:])
            ones_t = cpool.tile([1, P], mdt, tag="ones")
            nc.sync.dma_start(ones_t[:], ones_d[:])

            ident = cpool.tile([P, P], tdt, tag="ident")
            make_identity(nc, ident[:])

            iota_i = cpool.tile([P, P], i32, tag="iota_i")
            nc.gpsimd.iota(iota_i[:], pattern=[[1, P]], base=0,
                           channel_multiplier=0)
            iota_f = cpool.tile([P, P], f32, tag="iota_f")
            nc.vector.tensor_copy(iota_f[:], iota_i[:])

            srcidx_t = cpool.tile([P, ncols], i32, tag="srcidx")
            nc.sync.dma_start(srcidx_t[:], srcidx_d[:])
            dstloc_t = cpool.tile([P, ncols], f32, tag="dstloc")
            nc.sync.dma_start(dstloc_t[:], dstloc_d[:])
            degsrc_t = cpool.tile([P, ncols], f32, tag="degsrc")
            nc.sync.dma_start(degsrc_t[:], degsrc_d[:])
            degin_t = cpool.tile([P, bpc], f32, tag="degin")
            nc.sync.dma_start(degin_t[:], degin_d[:])

            # norms: n = sqrt(1/deg)
            nsrc_t = cpool.tile([P, ncols], f32, tag="nsrc")
            nc.vector.reciprocal(nsrc_t[:], degsrc_t[:])
            nc.scalar.activation(nsrc_t[:], nsrc_t[:], AF.Sqrt)
            ndst_t = cpool.tile([P, bpc], f32, tag="ndst")
            nc.vector.reciprocal(ndst_t[:], degin_t[:])
            nc.scalar.activation(ndst_t[:], ndst_t[:], AF.Sqrt)

            # perf-probe: one prologue-built S tile shared by all chunks
            sw_hoist = None
            if "sbuild-hoist" in ablate:
                sw_hoist = cpool.tile([P, P], gdt, tag="sw_hoist")
                nc.vector.tensor_scalar(
                    out=sw_hoist[:], in0=iota_f[:],
                    scalar1=dstloc_t[:, 0:1], scalar2=nsrc_t[:, 0:1],
                    op0=ALU.is_equal, op1=ALU.mult)
            g_hoist = None
            if "g-hoist" in ablate:
                g_hoist = cpool.tile([P, F_IN], gdt, tag="g_hoist")
                nc.gpsimd.indirect_dma_start(
                    out=g_hoist[:], out_offset=None, in_=x_d[:],
                    in_offset=bass.IndirectOffsetOnAxis(
                        ap=srcidx_t[:, 0:1], axis=0))

            # ---- main loop over node blocks
            for i in [i for _ in range(repeat) for i in range(bpc)]:
                p_agg = ps_agg.tile([P, F_IN], mybir.dt.float32, tag="agg")
                for c in range(C):
                    col = i * C + c
                    g = gpool.tile([P, F_IN], gdt, tag="g")
                    if "gather" not in ablate:
                        if n_queues <= 1:
                            nc.gpsimd.indirect_dma_start(
                                out=g[:], out_offset=None, in_=x_d[:],
                                in_offset=bass.IndirectOffsetOnAxis(
                                    ap=srcidx_t[:, col:col + 1], axis=0),
                            )
                        else:
                            _indirect_gather_q(
                                nc.gpsimd, g[:], x_d[:],
                                srcidx_t[:, col:col + 1], col % n_queues)
                    if "sbuild-hoist" in ablate:
                        sw = sw_hoist
                    else:
                        sw = spool.tile([P, P], gdt, tag="sw")
                        if "sbuild" not in ablate:
                            nc.vector.tensor_scalar(
                                out=sw[:], in0=iota_f[:],
                                scalar1=dstloc_t[:, col:col + 1],
                                scalar2=nsrc_t[:, col:col + 1],
                                op0=ALU.is_equal, op1=ALU.mult)
                    if "scatmm" not in ablate:
                        nc.tensor.matmul(
                            p_agg[:], lhsT=sw[:],
                            rhs=(g_hoist if "g-hoist" in ablate else g)[:],
                            start=(c == 0), stop=(c == C - 1))

                # agg * ndst -> SBUF
                agg_sb = apool.tile([P, F_IN], tdt, tag="agg_sb")
                if "aggcopy" not in ablate and "scatmm" not in ablate:
                    nc.scalar.activation(agg_sb[:], p_agg[:], AF.Copy,
                                         scale=ndst_t[:, i:i + 1])

                # transpose agg (feats onto partitions)
                p_tr = ps_t.tile([P, F_IN], tdt, tag="tr")
                aggT = tpool.tile([P, K_CH * P], mdt, tag="aggT")
                if "transpose" not in ablate:
                    for k in range(K_CH):
                        nc.tensor.transpose(
                            p_tr[:, k * P:(k + 1) * P],
                            in_=agg_sb[:, k * P:(k + 1) * P],
                            identity=ident[:])
                    for k in range(K_CH):
                        nc.vector.tensor_copy(aggT[:, k * P:(k + 1) * P],
                                              p_tr[:, k * P:(k + 1) * P])

                # GEMM + bias
                p_out = ps_out.tile([P, F_OUT], mybir.dt.float32, tag="out")
                if "gemm" not in ablate:
                    nc.tensor.matmul(p_out[:], lhsT=ones_t[:1, :],
                                     rhs=b_t[:1, :], start=True, stop=False)
                    for k in range(K_CH):
                        nc.tensor.matmul(
                            p_out[:], lhsT=aggT[:, k * P:(k + 1) * P],
                            rhs=w_t[:, k * F_OUT:(k + 1) * F_OUT],
                            start=False, stop=(k == K_CH - 1))

                out_sb = opool.tile([P, F_OUT], f32, tag="out_sb")
                if "gemm" not in ablate:
                    nc.scalar.activation(out_sb[:], p_out[:], AF.Relu)
                    nc.sync.dma_start(out_d[i * P:(i + 1) * P, :], out_sb[:])

    nc.compile()
    return nc


# ----------------------------------------------------------------------------
# numpy emulation of the device program (for logic validation)
# ----------------------------------------------------------------------------

def _emulate(x, W, b, pre):
    bpc, C = pre["bpc"], pre["C"]
    outs = []
    iota = np.arange(P, dtype=np.float32)
    for core in range(N_CORES):
        src_idx = pre["src_idx"][core]
        dstloc = pre["dstloc"][core]
        nsrc = np.sqrt(1.0 / pre["degsrc"][core])
        ndst = np.sqrt(1.0 / pre["degin"][core])
        out_core = np.empty((bpc * P, F_OUT), np.float32)
        for i in range(bpc):
            agg = np.zeros((P, F_IN), np.float32)
            for c in range(C):
                col = i * C + c
                g = x[src_idx[:, col]]
                sw = (iota[None, :] == dstloc[:, col:col + 1]) * \
                    nsrc[:, col:col + 1]
                agg += sw.T.astype(np.float32) @ g
            agg_sb = agg * ndst[:, i:i + 1]
            out_core[i * P:(i + 1) * P] = np.maximum(agg_sb @ W + b, 0.0)
        outs.append(out_core)
    return outs


# ----------------------------------------------------------------------------
# entry point
# ----------------------------------------------------------------------------

def _make_in_maps(x, W, b, pre, gather_dtype="f32r"):
    np_gdt = {"f32r": np.float32, "fp16": np.float16,
              "bf16": None}[gather_dtype]
    if np_gdt is None:
        import ml_dtypes
        np_gdt = ml_dtypes.bfloat16
    np_mdt = np.float32 if gather_dtype == "f32r" else np_gdt
    ones = np.ones((1, P), np_mdt)
    b_row = np.ascontiguousarray(b.reshape(1, F_OUT).astype(np_mdt))
    x = np.ascontiguousarray(x.astype(np_gdt))
    W = np.ascontiguousarray(W.astype(np_mdt))
    in_maps = []
    for core in range(N_CORES):
        in_maps.append({
            "x": x,
            "w": W,
            "b": b_row,
            "ones": ones,
            "src_idx": np.ascontiguousarray(pre["src_idx"][core]),
            "dstloc": np.ascontiguousarray(pre["dstloc"][core]),
            "degsrc": np.ascontiguousarray(pre["degsrc"][core]),
            "degin": np.ascontiguousarray(pre["degin"][core]),
        })
    return in_maps


def _assemble(outs, pre, n_nodes):
    full = np.concatenate(outs, axis=0)           # [npad, F_OUT]
    node_order = pre["node_order"]
    valid = node_order >= 0
    result = np.empty((n_nodes, F_OUT), np.float32)
    result[node_order[valid]] = full[valid]
    return result


def kernel(x, src, dst, W, b):
    x = np.asarray(x)
    src = np.asarray(src).astype(np.int64)
    dst = np.asarray(dst).astype(np.int64)
    W = np.asarray(W)
    b = np.asarray(b)
    n_nodes = x.shape[0]

    pre = _preprocess(src, dst, n_nodes)

    if os.environ.get("GNN_KERNEL_EMULATE"):
        outs = _emulate(x.astype(np.float32), W.astype(np.float32),
                        b.astype(np.float32), pre)
        return _assemble(outs, pre, n_nodes)

    from concourse import bass_utils

    gather_dtype = os.environ.get("GNN_GATHER_DTYPE", GATHER_DTYPE)
    key = (n_nodes, pre["bpc"], pre["C"], gather_dtype)
    if key not in _PROGRAM_CACHE:
        _PROGRAM_CACHE[key] = _build_program(
            n_nodes, pre["bpc"], pre["C"], gather_dtype=gather_dtype)
    nc = _PROGRAM_CACHE[key]

    in_maps = _make_in_maps(x, W, b, pre, gather_dtype=gather_dtype)
    res = bass_utils.run_bass_kernel_spmd(
        nc, in_maps, core_ids=list(range(N_CORES)))
    outs = [res.results[c]["out"] for c in range(N_CORES)]
    return _assemble(outs, pre, n_nodes)


# revision 32
# speedup vs baseline: 1.1734x; 1.1734x over previous
"""GraphConv (DGL norm='both' + relu) Trainium2 kernel, 8-core SPMD.

out = relu( D_dst^{-1/2} A D_src^{-1/2} X W + b )

Strategy (per sharding hint): nodes are partitioned across the 8 cores;
edges are partitioned by destination node so the segment-sum scatter is
device-local; x is replicated so source features are gathered directly
from HBM (the "all-gather" is done at input-distribution time); W/b are
replicated.

Device algorithm, per 128-node block:
  - indirect-DMA gather of the block's edge source rows H [128e, 512]
  - build the one-hot scatter matrix S_w[e, n] = (dstloc[e]==n)*nsrc[e]
    on DVE (iota + fused is_equal/mult)
  - PE: agg = S_w.T @ H accumulated over edge chunks  (the segment sum)
  - ACT: agg_sb = agg * ndst  (PSUM->SBUF copy with per-partition scale)
  - PE: 4x 128x128 transposes -> aggT (feats-on-partitions for the GEMM)
  - PE: out = aggT.T @ W + b (bias via a K=1 ones-matmul), ACT relu
Matmuls run in float32r (TF32-like, 4x the fp32 rate); DRAM tensors are
declared float32r and carry raw fp32 bytes (PE rounds internally).

Host does only index-space preprocessing: degree counts (bincount),
balanced node->block assignment, edge bucketing/padding, and the final
inverse permutation of the output rows.
"""

import heapq
import os
import sys

import numpy as np

sys.path.insert(0, "/opt/trn_rl_repo")

P = 128          # partitions / nodes per block
N_CORES = 8
F_IN = 512
F_OUT = 512
K_CH = F_IN // P  # 4 contraction chunks in the GEMM
GATHER_DTYPE = "f32r"  # "f32r" | "fp16" | "bf16"

_PROGRAM_CACHE = {}


# ----------------------------------------------------------------------------
# host-side graph preprocessing (index-space only)
# ----------------------------------------------------------------------------

def _preprocess(src, dst, n_nodes):
    """Balanced node->block assignment + edge bucketing.

    Returns dict with per-core index arrays and the node permutation.
    """
    E = src.shape[0]
    bpc = int(np.ceil(n_nodes / (N_CORES * P)))      # blocks per core
    nblocks = N_CORES * bpc
    npad = nblocks * P

    deg_out = np.bincount(src, minlength=n_nodes).astype(np.int64)
    deg_in = np.bincount(dst, minlength=n_nodes).astype(np.int64)

    # Greedy balanced assignment of nodes to blocks (minimize max block
    # in-edge count so every block needs the same number of edge chunks).
    order = np.argsort(-deg_in, kind="stable")
    block_of = np.empty(n_nodes, np.int64)
    slot_of = np.empty(n_nodes, np.int64)
    counts = np.zeros(nblocks, np.int64)
    heap = [(0, b) for b in range(nblocks)]
    heapq.heapify(heap)
    deg_in_l = deg_in.tolist()
    for n in order.tolist():
        load, b = heapq.heappop(heap)
        block_of[n] = b
        slot_of[n] = counts[b]
        counts[b] += 1
        if counts[b] < P:
            heapq.heappush(heap, (load + deg_in_l[n], b))

    node_order = np.full(npad, -1, np.int64)
    node_order[block_of * P + slot_of] = np.arange(n_nodes)

    # Edge bucketing by destination block.
    eblk = block_of[dst]
    ec = np.bincount(eblk, minlength=nblocks)
    C = max(1, int(np.ceil(ec.max() / P)))           # chunks per block
    ek = np.lexsort((src, eblk))                      # group by block, then src
    eblk_s = eblk[ek]
    starts = np.concatenate(([0], np.cumsum(ec)))
    pos = np.arange(E) - starts[eblk_s]
    chunk = pos // P
    part = pos % P
    core = eblk_s // bpc
    blk_in_core = eblk_s % bpc
    col = blk_in_core * C + chunk

    ncols = bpc * C
    src_idx = np.zeros((N_CORES, P, ncols), np.int32)
    dstloc = np.full((N_CORES, P, ncols), -1.0, np.float32)
    degsrc = np.ones((N_CORES, P, ncols), np.float32)
    src_idx[core, part, col] = src[ek].astype(np.int32)
    dstloc[core, part, col] = slot_of[dst[ek]].astype(np.float32)
    degsrc[core, part, col] = deg_out[src[ek]].astype(np.float32)

    # per-node in-degree, laid out [core, slot(partition), block]
    deg_in_pad = np.ones(npad, np.float32)
    valid = node_order >= 0
    d = deg_in[node_order[valid]]
    deg_in_pad[valid] = np.where(d > 0, d, 1).astype(np.float32)
    degin = np.transpose(deg_in_pad.reshape(N_CORES, bpc, P), (0, 2, 1)).copy()

    return dict(
        bpc=bpc, C=C, npad=npad, node_order=node_order,
        src_idx=src_idx, dstloc=dstloc, degsrc=degsrc, degin=degin,
    )


# ----------------------------------------------------------------------------
# device program
# ----------------------------------------------------------------------------

def _indirect_gather_q(eng, out, in_, offset_ap, queue):
    """nc.gpsimd.indirect_dma_start (gather form), with a SWDGE queue choice.

    Replicates bass.BassGpSimd.indirect_dma_start's lowering but emits the
    InstDMACopy on qPoolDynamic{queue} so gathers can spread across multiple
    SWDGE contexts.
    """
    import concourse.mybir as mybir

    out_ap = eng.lower_ap_dma(out, for_indirect_dma=True)
    in_ap = eng.lower_ap_dma(in_, for_indirect_dma=True)
    assert len(in_ap) == 1 and len(out_ap) == 1
    offset_l = eng.lower_ap_dma(offset_ap)
    assert len(offset_l) == 1
    in_ap.append(offset_l[0])

    ap_shape = in_.shape
    coef = 1
    for i in range(1, len(ap_shape)):
        coef *= ap_shape[i]
    in_ap[0].dynamic_ap_info = mybir.DynamicAccessPatternInfo(
        c=0,
        actual_ap=out.ap,
        indirect_dim_max_index=ap_shape[0],
        offset_expr=[
            mybir.DynamicAccessPatternOffsetExpr(
                coef=coef,
                aff_expr=mybir.DynamicAccessPatternOffsetExprAffExpr(
                    kind="IndirectArgId", arg_id=1),
            )
        ],
    )
    return eng.add_instruction(
        mybir.InstDMACopy(
            name=eng.bass.get_next_instruction_name(),
            queue=f"qPoolDynamic{queue or ''}",
            mode="Copy",
            ins=in_ap,
            outs=out_ap,
            oob_is_err=True,
            cce_op=mybir.AluOpType.bypass,
        )
    )


def _build_program(n_nodes, bpc, C, repeat=1, ablate=(), n_queues=1,
                   gather_dtype="f32r", bufs_g=12, aggt_act=False):
    import concourse.bass as bass
    import concourse.tile as tile
    from concourse import bacc, mybir
    from concourse.masks import make_identity

    ablate = set(ablate)

    f32 = mybir.dt.float32
    f32r = mybir.dt.float32r
    i32 = mybir.dt.int32
    AF = mybir.ActivationFunctionType
    ALU = mybir.AluOpType
    gdt = {"f32r": f32r, "fp16": mybir.dt.float16,
           "bf16": mybir.dt.bfloat16}[gather_dtype]
    # 16-bit gather -> run the whole matmul pipeline (W, agg, transposes) in
    # the same 16-bit dtype: FWL fast weight loads + 1 cyc/row transposes.
    mdt = gdt if gather_dtype != "f32r" else f32r
    tdt = gdt if gather_dtype != "f32r" else f32  # transpose dtype

    ncols = bpc * C

    nc = bacc.Bacc("TRN2", target_bir_lowering=False, debug=False,
                   num_devices=N_CORES, num_swdge_queues=max(1, n_queues))

    x_d = nc.dram_tensor("x", [n_nodes, F_IN], gdt, kind="ExternalInput").ap()
    w_d = nc.dram_tensor("w", [F_IN, F_OUT], mdt, kind="ExternalInput").ap()
    b_d = nc.dram_tensor("b", [1, F_OUT], mdt, kind="ExternalInput").ap()
    ones_d = nc.dram_tensor("ones", [1, P], mdt, kind="ExternalInput").ap()
    srcidx_d = nc.dram_tensor("src_idx", [P, ncols], i32, kind="ExternalInput").ap()
    dstloc_d = nc.dram_tensor("dstloc", [P, ncols], f32, kind="ExternalInput").ap()
    degsrc_d = nc.dram_tensor("degsrc", [P, ncols], f32, kind="ExternalInput").ap()
    degin_d = nc.dram_tensor("degin", [P, bpc], f32, kind="ExternalInput").ap()
    out_d = nc.dram_tensor("out", [bpc * P, F_OUT], f32, kind="ExternalOutput").ap()

    with tile.TileContext(nc) as tc:
        with (
            tc.tile_pool(name="const", bufs=1) as cpool,
            tc.tile_pool(name="gpool", bufs=bufs_g) as gpool,
            tc.tile_pool(name="spool", bufs=12) as spool,
            tc.tile_pool(name="apool", bufs=3) as apool,
            tc.tile_pool(name="tpool", bufs=8) as tpool,
            tc.tile_pool(name="opool", bufs=3) as opool,
            tc.tile_pool(name="ps_agg", bufs=2, space="PSUM") as ps_agg,
            tc.tile_pool(name="ps_t", bufs=2, space="PSUM") as ps_t,
            tc.tile_pool(name="ps_out", bufs=2, space="PSUM") as ps_out,
        ):
            # ---- prologue: constants and index arrays
            w_t = cpool.tile([P, K_CH * F_OUT], mdt, tag="w")
            for k in range(K_CH):
                nc.sync.dma_start(
                    w_t[:, k * F_OUT:(k + 1) * F_OUT],
                    w_d[k * P:(k + 1) * P, :])
            b_t = cpool.tile([1, F_OUT], mdt, tag="b")
            nc.sync.dma_start(b_t[:], b_d[:])
            ones_t = cpool.tile([1, P], mdt, tag="ones")
            nc.sync.dma_start(ones_t[:], ones_d[:])

            ident = cpool.tile([P, P], tdt, tag="ident")
            make_identity(nc, ident[:])

            iota_i = cpool.tile([P, P], i32, tag="iota_i")
            nc.gpsimd.iota(iota_i[:], pattern=[[1, P]], base=0,
                           channel_multiplier=0)
            iota_f = cpool.tile([P, P], f32, tag="iota_f")
            nc.vector.tensor_copy(iota_f[:], iota_i[:])

            srcidx_t = cpool.tile([P, ncols], i32, tag="srcidx")
            nc.sync.dma_start(srcidx_t[:], srcidx_d[:])
            dstloc_t = cpool.tile([P, ncols], f32, tag="dstloc")
            nc.sync.dma_start(dstloc_t[:], dstloc_d[:])
            degsrc_t = cpool.tile([P, ncols], f32, tag="degsrc")
            nc.sync.dma_start(degsrc_t[:], degsrc_d[:])
            degin_t = cpool.tile([P, bpc], f32, tag="degin")
            nc.sync.dma_start(degin_t[:], degin_d[:])

            # norms: n = sqrt(1/deg)
            nsrc_t = cpool.tile([P, ncols], f32, tag="nsrc")
            nc.vector.reciprocal(nsrc_t[:], degsrc_t[:])
            nc.scalar.activation(nsrc_t[:], nsrc_t[:], AF.Sqrt)
            ndst_t = cpool.tile([P, bpc], f32, tag="ndst")
            nc.vector.reciprocal(ndst_t[:], degin_t[:])
            nc.scalar.activation(ndst_t[:], ndst_t[:], AF.Sqrt)

            # perf-probe: one prologue-built S tile shared by all chunks
            sw_hoist = None
            if "sbuild-hoist" in ablate:
                sw_hoist = cpool.tile([P, P], gdt, tag="sw_hoist")
                nc.vector.tensor_scalar(
                    out=sw_hoist[:], in0=iota_f[:],
                    scalar1=dstloc_t[:, 0:1], scalar2=nsrc_t[:, 0:1],
                    op0=ALU.is_equal, op1=ALU.mult)
            g_hoist = None
            if "g-hoist" in ablate:
                g_hoist = cpool.tile([P, F_IN], gdt, tag="g_hoist")
                nc.gpsimd.indirect_dma_start(
                    out=g_hoist[:], out_offset=None, in_=x_d[:],
                    in_offset=bass.IndirectOffsetOnAxis(
                        ap=srcidx_t[:, 0:1], axis=0))

            # ---- main loop over node blocks
            for i in [i for _ in range(repeat) for i in range(bpc)]:
                p_agg = ps_agg.tile([P, F_IN], mybir.dt.float32, tag="agg")
                for c in range(C):
                    col = i * C + c
                    g = gpool.tile([P, F_IN], gdt, tag="g")
                    if "gather" not in ablate:
                        if n_queues <= 1:
                            nc.gpsimd.indirect_dma_start(
                                out=g[:], out_offset=None, in_=x_d[:],
                                in_offset=bass.IndirectOffsetOnAxis(
                                    ap=srcidx_t[:, col:col + 1], axis=0),
                            )
                        else:
                            _indirect_gather_q(
                                nc.gpsimd, g[:], x_d[:],
                                srcidx_t[:, col:col + 1], col % n_queues)
                    if "sbuild-hoist" in ablate:
                        sw = sw_hoist
                    else:
                        sw = spool.tile([P, P], gdt, tag="sw")
                        if "sbuild" not in ablate:
                            nc.vector.tensor_scalar(
                                out=sw[:], in0=iota_f[:],
                                scalar1=dstloc_t[:, col:col + 1],
                                scalar2=nsrc_t[:, col:col + 1],
                                op0=ALU.is_equal, op1=ALU.mult)
                    if "scatmm" not in ablate:
                        nc.tensor.matmul(
                            p_agg[:], lhsT=sw[:],
                            rhs=(g_hoist if "g-hoist" in ablate else g)[:],
                            start=(c == 0), stop=(c == C - 1))

                # agg * ndst -> SBUF
                agg_sb = apool.tile([P, F_IN], tdt, tag="agg_sb")
                if "aggcopy" not in ablate and "scatmm" not in ablate:
                    nc.scalar.activation(agg_sb[:], p_agg[:], AF.Copy,
                                         scale=ndst_t[:, i:i + 1])

                # transpose agg (feats onto partitions)
                p_tr = ps_t.tile([P, F_IN], tdt, tag="tr")
                aggT = tpool.tile([P, K_CH * P], mdt, tag="aggT")
                if "transpose" not in ablate:
                    for k in range(K_CH):
                        nc.tensor.transpose(
                            p_tr[:, k * P:(k + 1) * P],
                            in_=agg_sb[:, k * P:(k + 1) * P],
                            identity=ident[:])
                    for k in range(K_CH):
                        if aggt_act:
                            nc.scalar.activation(
                                aggT[:, k * P:(k + 1) * P],
                                p_tr[:, k * P:(k + 1) * P], AF.Copy)
                        else:
                            nc.vector.tensor_copy(aggT[:, k * P:(k + 1) * P],
                                                  p_tr[:, k * P:(k + 1) * P])

                # GEMM + bias
                p_out = ps_out.tile([P, F_OUT], mybir.dt.float32, tag="out")
                if "gemm" not in ablate:
                    nc.tensor.matmul(p_out[:], lhsT=ones_t[:1, :],
                                     rhs=b_t[:1, :], start=True, stop=False)
                    for k in range(K_CH):
                        nc.tensor.matmul(
                            p_out[:], lhsT=aggT[:, k * P:(k + 1) * P],
                            rhs=w_t[:, k * F_OUT:(k + 1) * F_OUT],
                            start=False, stop=(k == K_CH - 1))

                out_sb = opool.tile([P, F_OUT], f32, tag="out_sb")
                if "gemm" not in ablate:
                    nc.scalar.activation(out_sb[:], p_out[:], AF.Relu)
                    nc.sync.dma_start(out_d[i * P:(i + 1) * P, :], out_sb[:])

    nc.compile()
    return nc


# ----------------------------------------------------------------------------
# numpy emulation of the device program (for logic validation)
# ----------------------------------------------------------------------------

def _emulate(x, W, b, pre):
    bpc, C = pre["bpc"], pre["C"]
    outs = []
    iota = np.arange(P, dtype=np.float32)
    for core in range(N_CORES):
        src_idx = pre["src_idx"][core]
        dstloc = pre["dstloc"][core]
        nsrc = np.sqrt(1.0 / pre["degsrc"][core])
        ndst = np.sqrt(1.0 / pre["degin"][core])
        out_core = np.empty((bpc * P, F_OUT), np.float32)
        for i in range(bpc):
            agg = np.zeros((P, F_IN), np.float32)
            for c in range(C):
                col = i * C + c
                g = x[src_idx[:, col]]
                sw = (iota[None, :] == dstloc[:, col:col + 1]) * \
                    nsrc[:, col:col + 1]
                agg += sw.T.astype(np.float32) @ g
            agg_sb = agg * ndst[:, i:i + 1]
            out_core[i * P:(i + 1) * P] = np.maximum(agg_sb @ W + b, 0.0)
        outs.append(out_core)
    return outs


# ----------------------------------------------------------------------------
# entry point
# ----------------------------------------------------------------------------

def _make_in_maps(x, W, b, pre, gather_dtype="f32r"):
    np_gdt = {"f32r": np.float32, "fp16": np.float16,
              "bf16": None}[gather_dtype]
    if np_gdt is None:
        import ml_dtypes
        np_gdt = ml_dtypes.bfloat16
    np_mdt = np.float32 if gather_dtype == "f32r" else np_gdt
    ones = np.ones((1, P), np_mdt)
    b_row = np.ascontiguousarray(b.reshape(1, F_OUT).astype(np_mdt))
    x = np.ascontiguousarray(x.astype(np_gdt))
    W = np.ascontiguousarray(W.astype(np_mdt))
    in_maps = []
    for core in range(N_CORES):
        in_maps.append({
            "x": x,
            "w": W,
            "b": b_row,
            "ones": ones,
            "src_idx": np.ascontiguousarray(pre["src_idx"][core]),
            "dstloc": np.ascontiguousarray(pre["dstloc"][core]),
            "degsrc": np.ascontiguousarray(pre["degsrc"][core]),
            "degin": np.ascontiguousarray(pre["degin"][core]),
        })
    return in_maps


def _assemble(outs, pre, n_nodes):
    full = np.concatenate(outs, axis=0)           # [npad, F_OUT]
    node_order = pre["node_order"]
    valid = node_order >= 0
    result = np.empty((n_nodes, F_OUT), np.float32)
    result[node_order[valid]] = full[valid]
    return result


def kernel(x, src, dst, W, b):
    x = np.asarray(x)
    src = np.asarray(src).astype(np.int64)
    dst = np.asarray(dst).astype(np.int64)
    W = np.asarray(W)
    b = np.asarray(b)
    n_nodes = x.shape[0]

    pre = _preprocess(src, dst, n_nodes)

    if os.environ.get("GNN_KERNEL_EMULATE"):
        outs = _emulate(x.astype(np.float32), W.astype(np.float32),
                        b.astype(np.float32), pre)
        return _assemble(outs, pre, n_nodes)

    from concourse import bass_utils

    gather_dtype = os.environ.get("GNN_GATHER_DTYPE", GATHER_DTYPE)
    key = (n_nodes, pre["bpc"], pre["C"], gather_dtype)
    if key not in _PROGRAM_CACHE:
        _PROGRAM_CACHE[key] = _build_program(
            n_nodes, pre["bpc"], pre["C"], gather_dtype=gather_dtype)
    nc = _PROGRAM_CACHE[key]

    in_maps = _make_in_maps(x, W, b, pre, gather_dtype=gather_dtype)
    res = bass_utils.run_bass_kernel_spmd(
        nc, in_maps, core_ids=list(range(N_CORES)))
    outs = [res.results[c]["out"] for c in range(N_CORES)]
    return _assemble(outs, pre, n_nodes)
